# revision 11
# baseline (speedup 1.0000x reference)
"""Trainium2 Bass kernel for HandmadeConv2d.

Conv2d NCHW, valid padding, stride 1, no bias:
  x: (32, 128, 64, 64) f32, weights: (256, 128, 3, 3) f32 -> out: (32, 256, 62, 62) f32

Sharding: data-parallel over batch, 4 images per core across 8 NeuronCores;
weights replicated.

Per core the conv is computed as 9 accumulating matmuls per output tile:
  out[oc, (oh,ow)] += W[kh,kw][ic, oc].T @ x[ic, (oh+kh, ow+kw)]
with ic=128 as the PE contraction dim, oc split into 2 chunks of 128
(PSUM partition dim), and spatial tiled as 8 output rows x 62 cols = 496
moving-operand elements (<=512 fp32 limit, fits one PSUM bank).

All data preparation happens on the host: weights are pre-transposed to
[ic, kh*kw, oc] (so they DMA straight into the stationary-operand layout)
and, for the fp32r modes, operands are pre-rounded to the PE's fp32r
format (round-to-nearest-even keeping 11 mantissa bits) so the device
performs zero weight transposes and zero dtype casts.

Matmul dtype modes (BASS_CONV_MODE env var):
  fp32      - native fp32 matmul (4 cycles/row), bitwise-matches the jax
              reference on TRN2
  fp32r     - single-pass rounded fp32 (1 cycle/row), ~1.4e-4 rel err
  fp32rsplit- hi/lo fp32r decomposition, 3 matmuls, ~2e-7 rel err
  bf16split - hi/lo bf16 decomposition, 3 matmuls, ~5e-6 rel err
"""

import os
import warnings

warnings.filterwarnings("ignore")

import numpy as np

N_CORES = 8
NIMG = 4  # images per core
IC = 128
OC = 256
H = W = 64
OH = OW = 62
P = 128

MODE = os.environ.get("BASS_CONV_MODE", "fp32r")

_NC_CACHE = {}

# x row-bands (2-row halo) so first matmuls start after ~1/4 image is resident
BANDS = [(0, 18), (16, 18), (32, 18), (48, 16)]  # (row0, nrows)


def _row_groups():
    groups = []
    r = 0
    while r < OH:
        nr = min(8, OH - r)
        groups.append((r, nr))
        r += nr
    return groups


def round_fp32r(a):
    """Round fp32 to the PE's fp32r format: RNE keeping 11 mantissa bits.
    Matches the hardware's rounding (validated bit-level on TRN2)."""
    u = np.ascontiguousarray(a, dtype=np.float32).view(np.uint32)
    low = u & np.uint32(0xFFF)
    base = u & np.uint32(0xFFFFF000)
    lsb = (u >> np.uint32(12)) & np.uint32(1)
    up = (low > 0x800) | ((low == 0x800) & (lsb == 1))
    r = base + (up.astype(np.uint32) << np.uint32(12))
    return r.view(np.float32).reshape(a.shape)


def build_nc(mode):
    import concourse.bacc as bacc
    import concourse.mybir as mybir
    import concourse.tile as tile

    f32 = mybir.dt.float32
    if mode == "fp32":
        ddt = f32
    elif mode in ("fp32r", "fp32rsplit"):
        ddt = mybir.dt.float32r
    elif mode == "bf16split":
        ddt = mybir.dt.bfloat16
    else:
        raise ValueError(mode)
    split = mode in ("fp32rsplit", "bf16split")

    nc = bacc.Bacc("TRN2", target_bir_lowering=False, debug=False)
    xh = nc.dram_tensor("xh", [NIMG, IC, H, W], ddt, kind="ExternalInput")
    wh = nc.dram_tensor("wh", [IC, 9, OC], ddt, kind="ExternalInput")
    if split:
        xl = nc.dram_tensor("xl", [NIMG, IC, H, W], ddt, kind="ExternalInput")
        wl = nc.dram_tensor("wl", [IC, 9, OC], ddt, kind="ExternalInput")
    out = nc.dram_tensor("out", [NIMG, OC, OH, OW], f32, kind="ExternalOutput")

    groups = _row_groups()

    with tile.TileContext(nc) as tc:
        with (
            tc.tile_pool(name="wtiles", bufs=1) as wtiles,
            tc.tile_pool(name="xconv", bufs=8) as xconv,
            tc.tile_pool(name="osb", bufs=8) as osb,
            tc.tile_pool(name="psmm", bufs=8, space="PSUM") as psmm,
        ):
            # startup-ordered DMAs: first x band, then weights in 3 chunks
            # (first matmul only needs band 0 + the k=0..2 weight slice), so
            # the PE starts ~4us earlier than with one monolithic weight DMA.
            def load_bands(n, engine=None):
                eng = engine or nc.sync
                terms = []
                for b0, bn in BANDS:
                    bhi = xconv.tile([P, 18, W], ddt, tag="xbh")
                    eng.dma_start(bhi[:, :bn, :], xh[:][n, :, b0 : b0 + bn, :])
                    terms_b = [bhi]
                    if split:
                        blo = xconv.tile([P, 18, W], ddt, tag="xbl")
                        eng.dma_start(blo[:, :bn, :], xl[:][n, :, b0 : b0 + bn, :])
                        terms_b.append(blo)
                    terms.append(terms_b)
                return terms

            wt_hi = wtiles.tile([P, 9, OC], ddt, tag="wt_hi")
            if split:
                wt_lo = wtiles.tile([P, 9, OC], ddt, tag="wt_lo")

            # weight chunks spread across three engines' DMA queues so the
            # transfers run in parallel (startup critical path); image-0
            # bands issued concurrently from GpSimd's queue.
            w_engines = [nc.sync, nc.scalar, nc.gpsimd]
            for ki, k0 in enumerate(range(0, 9, 3)):
                eng = w_engines[ki]
                eng.dma_start(wt_hi[:, k0 : k0 + 3, :], wh[:][:, k0 : k0 + 3, :])
                if split:
                    eng.dma_start(wt_lo[:, k0 : k0 + 3, :], wl[:][:, k0 : k0 + 3, :])

            # PE pre-warm: dummy matmuls on a zeroed tile bridge the initial
            # DMA wait, so HAM un-throttles the PE clock (1.2->2.4 GHz) and
            # stays un-throttled until the first real matmul issues.
            warm = wtiles.tile([P, 256], mybir.dt.bfloat16, tag="warm")
            nc.gpsimd.memset(warm[:], 0.0)
            for _ in range(20):
                wps = psmm.tile([P, 8 * OW], mybir.dt.float32, tag="mm")
                nc.tensor.matmul(
                    wps[:, :256], warm[:, :P], warm[:, :256], start=True, stop=True
                )

            for n in range(NIMG):
                xb_terms = load_bands(n, engine=nc.scalar if n == 0 else None)

                for c in range(2):
                    for r0, nr in groups:
                        b = min(3, r0 // 16)
                        b0 = BANDS[b][0]
                        xts = xb_terms[b]
                        if split:
                            terms = [(wt_hi, xts[0]), (wt_hi, xts[1]), (wt_lo, xts[0])]
                        else:
                            terms = [(wt_hi, xts[0])]
                        ps_t = psmm.tile([P, 8 * OW], mybir.dt.float32, tag="mm")
                        nmm = len(terms) * 9
                        i = 0
                        for wt, xt in terms:
                            for k in range(9):
                                kh, kw = divmod(k, 3)
                                rr = r0 - b0 + kh
                                nc.tensor.matmul(
                                    ps_t[:, : nr * OW],
                                    wt[:, k, c * P : (c + 1) * P],
                                    xt[:, rr : rr + nr, kw : kw + OW],
                                    start=(i == 0),
                                    stop=(i == nmm - 1),
                                )
                                i += 1
                        ob = osb.tile([P, 8 * OW], mybir.dt.float32, tag="ob")
                        nc.any.tensor_copy(ob[:, : nr * OW], ps_t[:, : nr * OW])
                        nc.sync.dma_start(
                            out[:][n, c * P : (c + 1) * P, r0 : r0 + nr, :],
                            ob[:, : nr * OW].rearrange("p (r q) -> p r q", q=OW),
                        )

    nc.compile()
    return nc


def get_nc(mode=None):
    mode = mode or MODE
    if mode not in _NC_CACHE:
        _NC_CACHE[mode] = build_nc(mode)
    return _NC_CACHE[mode]


def _host_prep(x, weights, mode):
    """Host-side data prep: weight transpose to [ic, kh*kw, oc] plus
    per-mode rounding / hi-lo decomposition."""
    x = np.ascontiguousarray(np.asarray(x), dtype=np.float32)
    w = np.ascontiguousarray(np.asarray(weights), dtype=np.float32)
    wt = np.ascontiguousarray(w.transpose(1, 2, 3, 0)).reshape(IC, 9, OC)

    if mode == "fp32":
        return {"xh": x, "wh": wt}
    if mode == "fp32r":
        return {"xh": round_fp32r(x), "wh": round_fp32r(wt)}
    if mode == "fp32rsplit":
        xhi = round_fp32r(x)
        whi = round_fp32r(wt)
        return {
            "xh": xhi,
            "xl": round_fp32r(x - xhi),
            "wh": whi,
            "wl": round_fp32r(wt - whi),
        }
    if mode == "bf16split":
        import ml_dtypes

        bf = ml_dtypes.bfloat16
        xhi = x.astype(bf)
        whi = wt.astype(bf)
        xlo = (x - xhi.astype(np.float32)).astype(bf)
        wlo = (wt - whi.astype(np.float32)).astype(bf)
        return {"xh": xhi, "xl": xlo, "wh": whi, "wl": wlo}
    raise ValueError(mode)


def kernel(x, weights, _trace=False, _mode=None):
    from concourse.bass_utils import run_bass_kernel_spmd

    mode = _mode or MODE
    nc = get_nc(mode)
    tensors = _host_prep(x, weights, mode)
    in_maps = []
    for i in range(N_CORES):
        m = {}
        for k, v in tensors.items():
            m[k] = v[i * NIMG : (i + 1) * NIMG] if k.startswith("x") else v
        in_maps.append(m)
    res = run_bass_kernel_spmd(
        nc, in_maps, core_ids=list(range(N_CORES)), trace=_trace
    )
    out = np.concatenate([r["out"] for r in res.results], axis=0)
    if _trace:
        kernel.last_results = res
    return out


kernel.last_results = None
